# revision 17
# baseline (speedup 1.0000x reference)
"""GCNConv (COO SpMM aggregation + dense GEMM) on 8 Trainium2 NeuronCores.

  msgs = edge_vals[:, None] * x[edge_col]          # [E, 64] gather+scale
  agg  = segment_sum(msgs, edge_row, N)            # [N, 64] scatter-add
  out  = agg @ weight                              # [N, 64] GEMM

Sharding: destination-node sharding (each core owns a contiguous row slab and
all edges targeting it) -> zero collectives.

The throughput limit is SWDGE descriptor generation for the per-edge row
gather (~9.6 ns/descriptor on one Q7 pair).  The kernel splits the gather
calls across all 4 SWDGE queues (each queue runs on its own Q7 core pair),
parallelizing descriptor generation 4x.  Everything else is arranged to hide
under that wall:
  - x is stored bf16, feature-padded to 128 cols so each row is a 256B gather
    element; the gather output is directly the TensorE stationary operand.
  - the edge_vals scaling AND the destination one-hot are merged into a
    HOST-BUILT val-weighted bf16 one-hot (ohv[slot, r] = val if dest==r else
    0; padded slots all-zero), streamed from HBM.  The Vector engine does no
    per-edge work at all.
  - TensorE per 128-edge chunk (bf16): psum_aggT[64, 128 rows] +=
    msgs[:, :64].T @ ohv  -- transposed aggregates directly, no PE transpose.
  - Activation engine copies psum_aggT -> aggT (bf16), and the per-supergroup
    W GEMM outT[64, 896] = W.T @ aggT runs as 2 matmuls + ACT copy + one
    contiguous output DMA; host scatters rows back.

Host-side prep minimizes padded gather slots:
  - x is split into 4 unequal quarters (int16 gather indices), sized so each
    (block, quarter) edge-group mean sits well below a multiple of 128.
  - each core's 12544 rows are bin-packed into 98 blocks of 128 rows,
    balancing all 4 per-quarter degree sums; the row permutation is undone
    on the host at the end.
"""

import os
import sys

import numpy as np

if "/opt/trn_rl_repo" not in sys.path:
    sys.path.insert(0, "/opt/trn_rl_repo")

import ml_dtypes

# ---------------------------------------------------------------- constants
N = 100000
E = 1600000
D = 64
DP = 128             # padded feature count (256B bf16 gather elements)
CORES = 8
RPC = 12544          # rows per core (8*12544 = 100352 >= N)
BLOCKS = RPC // 128  # 98 dest blocks per core
Q = 4
QS = np.array([0, 30134, 54243, 78352, 100352], dtype=np.int64)  # quarter bounds
CAPQ = np.array([640, 512, 512, 512], dtype=np.int64)  # packing targets
G = 7                # dest blocks per gather super-group (98 = 14*7)
NGROUPS = BLOCKS // G

LAST_EXEC_TIME_NS = None
_CACHE = {}


def _pack_rows(deg):
    """Assign RPC rows (deg: [RPC, 4] per-quarter degrees) to BLOCKS blocks
    of 128, balancing all 4 quarter sums against the CAPQ targets.  Greedy
    rounds (one row per block per round) + peak-shaving swap repair.
    Returns perm_local[pos] = row, where pos = block*128 + slot."""
    order = np.argsort(-deg.sum(1), kind="stable")
    cur = np.zeros((BLOCKS, Q), np.float64)
    capf = CAPQ.astype(np.float64)
    blk_of = np.empty(RPC, np.int64)
    for rnd in range(128):
        batch = order[rnd * BLOCKS : (rnd + 1) * BLOCKS]
        bscore = (deg[batch] / capf).max(1)
        bo = batch[np.argsort(-bscore, kind="stable")]
        load = (cur / capf).max(1)
        blko = np.argsort(load, kind="stable")
        cur[blko] += deg[bo]
        blk_of[bo] = blko
    # repair: swap the heaviest row (in the hottest quarter) of the hottest
    # block with a light row of the coolest block
    loadi = np.zeros((BLOCKS, Q), np.int64)
    np.add.at(loadi, blk_of, deg)
    rows_in = [list(np.where(blk_of == b)[0]) for b in range(BLOCKS)]
    for _ in range(4000):
        nl = loadi / capf
        b, q = np.unravel_index(np.argmax(nl), nl.shape)
        b, q = int(b), int(q)
        if nl[b, q] <= 1.0:
            break
        cand = max(rows_in[b], key=lambda r: deg[r, q])
        tgt = int(np.argmin(nl[:, q] + (np.arange(BLOCKS) == b) * 10))
        cand2 = min(rows_in[tgt], key=lambda r: deg[r, q])
        loadi[b] += deg[cand2] - deg[cand]
        loadi[tgt] += deg[cand] - deg[cand2]
        rows_in[b].remove(cand)
        rows_in[b].append(cand2)
        rows_in[tgt].remove(cand2)
        rows_in[tgt].append(cand)
    perm_local = np.empty(RPC, np.int64)
    for b in range(BLOCKS):
        for j, r in enumerate(rows_in[b]):
            perm_local[b * 128 + j] = r
    return perm_local


# ---------------------------------------------------------------- host prep
def _prep(x, weight, edge_vals, edge_row, edge_col):
    e_row = np.asarray(edge_row, dtype=np.int64)
    e_col = np.asarray(edge_col, dtype=np.int64)
    ev = np.asarray(edge_vals, dtype=np.float32)
    x = np.asarray(x, dtype=np.float32)
    weight = np.asarray(weight, dtype=np.float32)
    ne = e_row.shape[0]
    NPAD = CORES * RPC

    qq = np.searchsorted(QS, e_col, side="right") - 1
    lidx = (e_col - QS[qq]).astype(np.int16)

    # per-row per-quarter degrees -> per-core packing permutation
    deg_flat = np.bincount(e_row * Q + qq, minlength=NPAD * Q).reshape(NPAD, Q)
    perm = np.empty((CORES, RPC), np.int64)      # perm[k, pos] = global row
    pos_of_row = np.empty(NPAD, np.int64)        # core-local position
    for k in range(CORES):
        pl = _pack_rows(deg_flat[k * RPC : (k + 1) * RPC])
        perm[k] = k * RPC + pl
        pos_of_row[perm[k]] = np.arange(RPC)

    core = e_row // RPC
    pos = pos_of_row[e_row]
    blk = pos // 128
    dest = (pos % 128).astype(np.int16)

    # group counts -> per-quarter chunk counts (global static)
    gkey = (core * BLOCKS + blk) * Q + qq
    counts = np.bincount(gkey, minlength=CORES * BLOCKS * Q)
    cmax = counts.reshape(CORES * BLOCKS, Q).max(axis=0)
    Cq = np.maximum(1, -(-cmax // 128))          # [Q] chunks per group
    SLq = Cq * 128
    SLOTSB = int(SLq.sum())                      # slots per block
    NCH = int(Cq.sum())                          # chunk-columns per block
    qslotoff = np.concatenate([[0], np.cumsum(SLq)[:-1]])

    order = np.argsort(gkey, kind="stable")
    NGK = CORES * BLOCKS * Q
    starts = np.zeros(NGK, np.int64)
    starts[1:] = np.cumsum(counts)[:-1]
    gsort = gkey[order]
    rank = np.arange(ne, dtype=np.int64) - starts[gsort]
    cb = gsort // Q
    qs = gsort % Q
    slot = cb * SLOTSB + qslotoff[qs] + rank

    NSLOT = CORES * BLOCKS * SLOTSB
    idx_flat = np.zeros(NSLOT, np.int16)          # pad gathers row 0
    dst_flat = np.full(NSLOT, -1, np.int16)       # pad -> all-zero onehot col
    val_flat = np.zeros(NSLOT, np.float32)        # pad scales to 0
    idx_flat[slot] = lidx[order]
    dst_flat[slot] = dest[order]
    val_flat[slot] = ev[order]

    slots = idx_flat.reshape(CORES, NGROUPS, G, SLOTSB)
    dsts = dst_flat.reshape(CORES, NGROUPS, G, SLOTSB)
    vals = val_flat.reshape(CORES, NGROUPS, G, SLOTSB)

    # gather idx per call (g, q): [G*SLq] block-major; wrap to [128, ./16]
    gi_parts = []
    for q in range(Q):
        arr = slots[:, :, :, qslotoff[q] : qslotoff[q] + SLq[q]]
        arr = np.ascontiguousarray(arr).reshape(CORES, NGROUPS, G * int(SLq[q]))
        w16 = arr.reshape(CORES, NGROUPS, -1, 16)
        w16 = np.moveaxis(w16, 3, 2)             # [C, NGR, 16, CALLE/16]
        gi_parts.append(np.tile(w16, (1, 1, 8, 1)))
    gidx = np.ascontiguousarray(np.concatenate(gi_parts, axis=3))

    # host-built val-weighted one-hot, chunk-column layout
    # [C, NGR, 128, G*NCH, 128]: column (lb, q, c) = lb*NCH + qchunkoff[q] + c,
    # ohv[., ., p, col, r] = val[slot] if dest[slot] == r else 0
    def to_cols(a):
        parts = []
        for q in range(Q):
            seg = a[:, :, :, qslotoff[q] : qslotoff[q] + SLq[q]]
            parts.append(
                np.ascontiguousarray(seg).reshape(
                    CORES, NGROUPS, G, int(Cq[q]), 128
                )
            )
        cols = np.concatenate(parts, axis=3)      # [C, NGR, G, NCH, 128]
        cols = cols.reshape(CORES, NGROUPS, G * NCH, 128)
        return np.ascontiguousarray(np.moveaxis(cols, 3, 2))

    dcol = to_cols(dsts)                          # [C, NGR, 128, G*NCH] int16
    vcol = to_cols(vals).astype(ml_dtypes.bfloat16)
    ohv = np.zeros((CORES, NGROUPS, 128, G * NCH, 128), ml_dtypes.bfloat16)
    np.put_along_axis(
        ohv, np.clip(dcol, 0, 127)[..., None].astype(np.int64),
        np.where(dcol >= 0, vcol, ml_dtypes.bfloat16(0))[..., None], axis=-1,
    )

    x_pad = np.zeros((int(QS[-1]), DP), ml_dtypes.bfloat16)
    x_pad[:N, :D] = x.astype(ml_dtypes.bfloat16)

    in_maps = []
    for k in range(CORES):
        in_maps.append(
            {
                "xq": x_pad,
                "w": np.ascontiguousarray(weight).astype(ml_dtypes.bfloat16),
                "gidx": np.ascontiguousarray(gidx[k]),
                "ohv": ohv[k],
            }
        )
    return in_maps, tuple(int(c) for c in Cq), perm


# ------------------------------------------------------------- bass program
def _build(Cq):
    import concourse.bacc as bacc
    import concourse.mybir as mybir
    import concourse.tile as tile

    f32 = mybir.dt.float32
    bf16 = mybir.dt.bfloat16
    i16 = mybir.dt.int16
    SLq = [c * 128 for c in Cq]
    NCH = sum(Cq)
    qchunkoff = [0]
    for c in Cq[:-1]:
        qchunkoff.append(qchunkoff[-1] + c)
    CALLE = [G * sl for sl in SLq]
    off16 = [0]
    for c in CALLE:
        off16.append(off16[-1] + c // 16)
    TOT16 = off16[-1]
    GR = G * 128                                  # rows per supergroup

    nc = bacc.Bacc(
        "TRN2",
        target_bir_lowering=False,
        debug=False,
        num_devices=CORES,
        num_swdge_queues=4,
    )
    NX = int(QS[-1])
    x_d = nc.dram_tensor("xq", [NX, DP], bf16, kind="ExternalInput")
    w_d = nc.dram_tensor("w", [D, D], bf16, kind="ExternalInput")
    gidx_d = nc.dram_tensor("gidx", [NGROUPS, 128, TOT16], i16, kind="ExternalInput")
    ohv_d = nc.dram_tensor(
        "ohv", [NGROUPS, 128, G * NCH, 128], bf16, kind="ExternalInput"
    )
    outT_d = nc.dram_tensor("outT", [D, RPC], f32, kind="ExternalOutput")

    with tile.TileContext(nc) as tc:
        with (
            tc.tile_pool(name="const", bufs=1) as cpool,
            tc.tile_pool(name="io", bufs=3) as iopool,
            tc.tile_pool(name="oh", bufs=2) as ohpool,
            tc.tile_pool(name="agg", bufs=2) as aggpool,
            tc.tile_pool(name="outsb", bufs=2) as opool,
            tc.tile_pool(name="pa", bufs=4, space="PSUM") as papool,
            tc.tile_pool(name="po", bufs=2, space="PSUM") as popool,
        ):
            w_sb = cpool.tile([D, D], bf16, name="w_sb")
            nc.scalar.dma_start(out=w_sb[:], in_=w_d[:])

            # persistent triple-buffered msgs tiles (gather fills every slot;
            # idx pads gather row 0, so contents are always finite; padded
            # slots have all-zero one-hot columns so they contribute nothing).
            # 3 buffers so group g's gathers only wait on the matmuls of
            # group g-3 -- two full group-periods of slack
            NB = 3
            msgs_t = [
                [
                    cpool.tile([128, G, Cq[q], DP], bf16, name=f"msgs{bi}_{q}")
                    for q in range(Q)
                ]
                for bi in range(NB)
            ]

            def emit_block(b, oh_t, aggT_g, rhs_fn):
                # aggT[64, 128 rows] += msgs_chunk[:, :64].T @ ohv_chunk
                lb = b % G
                pa = papool.tile([D, 128], f32, tag="pa", name=f"pa{b}")
                i = 0
                for q in range(Q):
                    for c in range(Cq[q]):
                        nc.tensor.matmul(
                            pa[:],
                            rhs_fn(q, c),
                            oh_t[:, lb * NCH + qchunkoff[q] + c, :],
                            start=(i == 0),
                            stop=(i == NCH - 1),
                        )
                        i += 1
                nc.scalar.copy(aggT_g[:, lb * 128 : (lb + 1) * 128], pa[:])

            def emit_group_out(g, aggT_g):
                # outT[64, GR] = W.T @ aggT slice; 2 matmuls (PSUM bank <=512)
                ot = opool.tile([D, GR], f32, tag="ot", name=f"ot{g}")
                for j, (o0, nn) in enumerate(((0, 512), (512, GR - 512))):
                    po = popool.tile([D, nn], f32, tag=f"po{j}", name=f"po{g}_{j}")
                    nc.tensor.matmul(
                        po[:],
                        w_sb[:],
                        aggT_g[:, o0 : o0 + nn],
                        start=True,
                        stop=True,
                    )
                    nc.scalar.copy(ot[:, o0 : o0 + nn], po[:])
                # scalar queue: keeps the sync queue a pure idx stream, so
                # the gathers' idx waits never count a slow DMA on a shared
                # DMAHW sem lane
                nc.scalar.dma_start(
                    out=outT_d[:, g * GR : (g + 1) * GR], in_=ot[:]
                )

            def load_io(g):
                # idx on the sync queue; the one-hot in 7 per-block pieces on
                # the (otherwise idle) Activation queue.  One big 1.95MB oh
                # DMA stalls the gathers ~18us/group: the Tile DMA-completion
                # sem lanes (DMAHW0-7) are shared round-robin across HWDGE
                # DMAs, so the gathers' idx wait transitively counts the oh
                # completion.  Small pieces complete in ~1us each, so the
                # false coupling costs nothing.
                idx_t = iopool.tile([128, TOT16], i16, tag="idx", name=f"idx{g}")
                oh_t = ohpool.tile(
                    [128, G * NCH, 128], bf16, tag="oh", name=f"oh{g}"
                )
                nc.sync.dma_start(out=idx_t[:], in_=gidx_d[g])
                for lb in range(G):
                    nc.scalar.dma_start(
                        out=oh_t[:, lb * NCH : (lb + 1) * NCH, :],
                        in_=ohv_d[g, :, lb * NCH : (lb + 1) * NCH, :],
                    )
                return idx_t, oh_t

            io_next = load_io(0)
            for g in range(NGROUPS):
                idx_t, oh_t = io_next
                if g + 1 < NGROUPS:
                    io_next = load_io(g + 1)
                aggT_g = aggpool.tile([D, GR], bf16, tag="aggT", name=f"aggT{g}")

                if g < NGROUPS - 1:
                    msgs = msgs_t[g % NB]
                    for q in range(Q):
                        m = msgs[q]
                        nc.gpsimd.dma_gather(
                            m[:].rearrange("p g c d -> p (g c) d"),
                            x_d[int(QS[q]) : int(QS[q + 1]), :],
                            idx_t[:, off16[q] : off16[q + 1]],
                            CALLE[q],
                            CALLE[q],
                            DP,
                            # single_packet=True needs the whole call inside
                            # the 1024-desc SWDGE ring -> crash on big calls
                            single_packet=False,
                            # round-robin the 4 SWDGE queues: each runs on its
                            # own Q7 core pair, so desc-gen parallelizes 4x
                            queue_num=(g + q) % 4,
                        )
                    for lb in range(G):
                        b = g * G + lb
                        emit_block(
                            b, oh_t, aggT_g,
                            lambda q, c, _m=msgs, _lb=lb: _m[q][:, _lb, c, :D],
                        )
                    emit_group_out(g, aggT_g)
                else:
                    # taper the final supergroup: per-block calls into
                    # dedicated ping-pong tiles so each block's compute
                    # overlaps the next block's gather, and the kernel tail
                    # is one block rather than a whole supergroup
                    for lb in range(G):
                        # alternate tiles 1/2 (tile 0 is still owned by the
                        # just-finished group 12); distinct tiles let block
                        # lb+1's gather overlap block lb's matmuls even with
                        # whole-tile dependency tracking
                        msgs = msgs_t[1 + lb % 2]
                        for q in range(Q):
                            nc.gpsimd.dma_gather(
                                msgs[q][:, lb, :, :],
                                x_d[int(QS[q]) : int(QS[q + 1]), :],
                                idx_t[
                                    :,
                                    off16[q]
                                    + lb * (SLq[q] // 16) : off16[q]
                                    + (lb + 1) * (SLq[q] // 16),
                                ],
                                SLq[q],
                                SLq[q],
                                DP,
                                single_packet=False,
                                # queue 0 carried group 12's big call and
                                # drains last; start the taper on queue 1
                                queue_num=(1 + lb + q) % 4,
                            )
                        b = g * G + lb
                        emit_block(
                            b, oh_t, aggT_g,
                            lambda q, c, _m=msgs, _lb=lb: _m[q][:, _lb, c, :D],
                        )
                        # per-block W GEMM + output: the kernel tail after the
                        # last gather is one 128-row block, not a whole group
                        po = papool.tile(
                            [D, 128], f32, tag="pa", name=f"pob{lb}"
                        )
                        nc.tensor.matmul(
                            po[:],
                            w_sb[:],
                            aggT_g[:, lb * 128 : (lb + 1) * 128],
                            start=True,
                            stop=True,
                        )
                        otb = opool.tile([D, 128], f32, tag="otb", name=f"otb{lb}")
                        nc.scalar.copy(otb[:], po[:])
                        nc.scalar.dma_start(
                            out=outT_d[:, b * 128 : (b + 1) * 128], in_=otb[:]
                        )

    nc.compile()
    return nc


# ----------------------------------------------------------------- kernel()
def _ensure_ntff_hook():
    """Provide antenv.axon_hooks (absent in this image) so that
    run_bass_kernel_spmd's BASS_TRACE path can register the axon NTFF
    profiler instead of crashing on import."""
    try:
        import antenv.axon_hooks  # noqa: F401

        return
    except ImportError:
        pass
    import types

    import antenv

    mod = types.ModuleType("antenv.axon_hooks")
    holder = {"hook": None}
    mod.set_axon_ntff_profile_hook = lambda h: holder.__setitem__("hook", h)
    mod.get_axon_ntff_profile_hook = lambda: holder["hook"]
    sys.modules["antenv.axon_hooks"] = mod
    antenv.axon_hooks = mod
    try:
        from trn_agent_boot.trn_boot import _ntff_profile_via_ctypes

        mod.set_axon_ntff_profile_hook(
            _ntff_profile_via_ctypes("/opt/axon/libaxon_pjrt.so")
        )
    except Exception:
        pass


def kernel(x, weight, edge_vals, edge_row, edge_col):
    global LAST_EXEC_TIME_NS
    from concourse.bass_utils import run_bass_kernel_spmd

    if os.environ.get("BASS_TRACE"):
        _ensure_ntff_hook()

    in_maps, Cq, perm = _prep(x, weight, edge_vals, edge_row, edge_col)
    if Cq not in _CACHE:
        _CACHE[Cq] = _build(Cq)
    nc = _CACHE[Cq]

    res = run_bass_kernel_spmd(nc, in_maps, list(range(CORES)))
    LAST_EXEC_TIME_NS = res.exec_time_ns

    out = np.empty((CORES * RPC, D), np.float32)
    for k in range(CORES):
        out[perm[k]] = res.results[k]["outT"].T
    return np.ascontiguousarray(out[:N])


# revision 18
# speedup vs baseline: 1.0236x; 1.0236x over previous
"""GCNConv (COO SpMM aggregation + dense GEMM) on 8 Trainium2 NeuronCores.

  msgs = edge_vals[:, None] * x[edge_col]          # [E, 64] gather+scale
  agg  = segment_sum(msgs, edge_row, N)            # [N, 64] scatter-add
  out  = agg @ weight                              # [N, 64] GEMM

Sharding: destination-node sharding (each core owns a contiguous row slab and
all edges targeting it) -> zero collectives.

The throughput limit is SWDGE descriptor generation for the per-edge row
gather (~9.6 ns/descriptor on one Q7 pair).  The kernel splits the gather
calls across all 4 SWDGE queues (each queue runs on its own Q7 core pair),
parallelizing descriptor generation 4x.  Everything else is arranged to hide
under that wall:
  - x is stored bf16, feature-padded to 128 cols so each row is a 256B gather
    element; the gather output is directly the TensorE stationary operand.
  - the edge_vals scaling AND the destination one-hot are merged into a
    HOST-BUILT val-weighted bf16 one-hot (ohv[slot, r] = val if dest==r else
    0; padded slots all-zero), streamed from HBM.  The Vector engine does no
    per-edge work at all.
  - TensorE per 128-edge chunk (bf16): psum_aggT[64, 128 rows] +=
    msgs[:, :64].T @ ohv  -- transposed aggregates directly, no PE transpose.
  - Activation engine copies psum_aggT -> aggT (bf16), and the per-supergroup
    W GEMM outT[64, 896] = W.T @ aggT runs as 2 matmuls + ACT copy + one
    contiguous output DMA; host scatters rows back.

Host-side prep minimizes padded gather slots:
  - x is split into 4 unequal quarters (int16 gather indices), sized so each
    (block, quarter) edge-group mean sits well below a multiple of 128.
  - each core's 12544 rows are bin-packed into 98 blocks of 128 rows,
    balancing all 4 per-quarter degree sums; the row permutation is undone
    on the host at the end.
"""

import os
import sys

import numpy as np

if "/opt/trn_rl_repo" not in sys.path:
    sys.path.insert(0, "/opt/trn_rl_repo")

import ml_dtypes

# ---------------------------------------------------------------- constants
N = 100000
E = 1600000
D = 64
DP = 128             # padded feature count (256B bf16 gather elements)
CORES = 8
RPC = 12544          # rows per core (8*12544 = 100352 >= N)
BLOCKS = RPC // 128  # 98 dest blocks per core
Q = 4
QS = np.array([0, 30134, 54243, 78352, 100352], dtype=np.int64)  # quarter bounds
CAPQ = np.array([640, 512, 512, 512], dtype=np.int64)  # packing targets
G = 7                # dest blocks per gather super-group (98 = 14*7)
NGROUPS = BLOCKS // G

LAST_EXEC_TIME_NS = None
_CACHE = {}


def _pack_rows(deg):
    """Assign RPC rows (deg: [RPC, 4] per-quarter degrees) to BLOCKS blocks
    of 128, balancing all 4 quarter sums against the CAPQ targets.  Greedy
    rounds (one row per block per round) + peak-shaving swap repair.
    Returns perm_local[pos] = row, where pos = block*128 + slot."""
    order = np.argsort(-deg.sum(1), kind="stable")
    cur = np.zeros((BLOCKS, Q), np.float64)
    capf = CAPQ.astype(np.float64)
    blk_of = np.empty(RPC, np.int64)
    for rnd in range(128):
        batch = order[rnd * BLOCKS : (rnd + 1) * BLOCKS]
        bscore = (deg[batch] / capf).max(1)
        bo = batch[np.argsort(-bscore, kind="stable")]
        load = (cur / capf).max(1)
        blko = np.argsort(load, kind="stable")
        cur[blko] += deg[bo]
        blk_of[bo] = blko
    # repair: swap the heaviest row (in the hottest quarter) of the hottest
    # block with a light row of the coolest block
    loadi = np.zeros((BLOCKS, Q), np.int64)
    np.add.at(loadi, blk_of, deg)
    rows_in = [list(np.where(blk_of == b)[0]) for b in range(BLOCKS)]
    for _ in range(4000):
        nl = loadi / capf
        b, q = np.unravel_index(np.argmax(nl), nl.shape)
        b, q = int(b), int(q)
        if nl[b, q] <= 1.0:
            break
        cand = max(rows_in[b], key=lambda r: deg[r, q])
        tgt = int(np.argmin(nl[:, q] + (np.arange(BLOCKS) == b) * 10))
        cand2 = min(rows_in[tgt], key=lambda r: deg[r, q])
        loadi[b] += deg[cand2] - deg[cand]
        loadi[tgt] += deg[cand] - deg[cand2]
        rows_in[b].remove(cand)
        rows_in[b].append(cand2)
        rows_in[tgt].remove(cand2)
        rows_in[tgt].append(cand)
    perm_local = np.empty(RPC, np.int64)
    for b in range(BLOCKS):
        for j, r in enumerate(rows_in[b]):
            perm_local[b * 128 + j] = r
    return perm_local


# ---------------------------------------------------------------- host prep
def _prep(x, weight, edge_vals, edge_row, edge_col):
    e_row = np.asarray(edge_row, dtype=np.int64)
    e_col = np.asarray(edge_col, dtype=np.int64)
    ev = np.asarray(edge_vals, dtype=np.float32)
    x = np.asarray(x, dtype=np.float32)
    weight = np.asarray(weight, dtype=np.float32)
    ne = e_row.shape[0]
    NPAD = CORES * RPC

    qq = np.searchsorted(QS, e_col, side="right") - 1
    lidx = (e_col - QS[qq]).astype(np.int16)

    # per-row per-quarter degrees -> per-core packing permutation
    deg_flat = np.bincount(e_row * Q + qq, minlength=NPAD * Q).reshape(NPAD, Q)
    perm = np.empty((CORES, RPC), np.int64)      # perm[k, pos] = global row
    pos_of_row = np.empty(NPAD, np.int64)        # core-local position
    for k in range(CORES):
        pl = _pack_rows(deg_flat[k * RPC : (k + 1) * RPC])
        perm[k] = k * RPC + pl
        pos_of_row[perm[k]] = np.arange(RPC)

    core = e_row // RPC
    pos = pos_of_row[e_row]
    blk = pos // 128
    dest = (pos % 128).astype(np.int16)

    # group counts -> per-quarter chunk counts (global static)
    gkey = (core * BLOCKS + blk) * Q + qq
    counts = np.bincount(gkey, minlength=CORES * BLOCKS * Q)
    cmax = counts.reshape(CORES * BLOCKS, Q).max(axis=0)
    Cq = np.maximum(1, -(-cmax // 128))          # [Q] chunks per group
    SLq = Cq * 128
    SLOTSB = int(SLq.sum())                      # slots per block
    NCH = int(Cq.sum())                          # chunk-columns per block
    qslotoff = np.concatenate([[0], np.cumsum(SLq)[:-1]])

    order = np.argsort(gkey, kind="stable")
    NGK = CORES * BLOCKS * Q
    starts = np.zeros(NGK, np.int64)
    starts[1:] = np.cumsum(counts)[:-1]
    gsort = gkey[order]
    rank = np.arange(ne, dtype=np.int64) - starts[gsort]
    cb = gsort // Q
    qs = gsort % Q
    slot = cb * SLOTSB + qslotoff[qs] + rank

    NSLOT = CORES * BLOCKS * SLOTSB
    idx_flat = np.zeros(NSLOT, np.int16)          # pad gathers row 0
    dst_flat = np.full(NSLOT, -1, np.int16)       # pad -> all-zero onehot col
    val_flat = np.zeros(NSLOT, np.float32)        # pad scales to 0
    idx_flat[slot] = lidx[order]
    dst_flat[slot] = dest[order]
    val_flat[slot] = ev[order]

    slots = idx_flat.reshape(CORES, NGROUPS, G, SLOTSB)
    dsts = dst_flat.reshape(CORES, NGROUPS, G, SLOTSB)
    vals = val_flat.reshape(CORES, NGROUPS, G, SLOTSB)

    # gather idx per call (g, q): [G*SLq] block-major; wrap to [128, ./16]
    gi_parts = []
    for q in range(Q):
        arr = slots[:, :, :, qslotoff[q] : qslotoff[q] + SLq[q]]
        arr = np.ascontiguousarray(arr).reshape(CORES, NGROUPS, G * int(SLq[q]))
        w16 = arr.reshape(CORES, NGROUPS, -1, 16)
        w16 = np.moveaxis(w16, 3, 2)             # [C, NGR, 16, CALLE/16]
        gi_parts.append(np.tile(w16, (1, 1, 8, 1)))
    gidx = np.ascontiguousarray(np.concatenate(gi_parts, axis=3))

    # host-built val-weighted one-hot, chunk-column layout
    # [C, NGR, 128, G*NCH, 128]: column (lb, q, c) = lb*NCH + qchunkoff[q] + c,
    # ohv[., ., p, col, r] = val[slot] if dest[slot] == r else 0
    def to_cols(a):
        parts = []
        for q in range(Q):
            seg = a[:, :, :, qslotoff[q] : qslotoff[q] + SLq[q]]
            parts.append(
                np.ascontiguousarray(seg).reshape(
                    CORES, NGROUPS, G, int(Cq[q]), 128
                )
            )
        cols = np.concatenate(parts, axis=3)      # [C, NGR, G, NCH, 128]
        cols = cols.reshape(CORES, NGROUPS, G * NCH, 128)
        return np.ascontiguousarray(np.moveaxis(cols, 3, 2))

    dcol = to_cols(dsts)                          # [C, NGR, 128, G*NCH] int16
    vcol = to_cols(vals).astype(ml_dtypes.bfloat16)
    ohv = np.zeros((CORES, NGROUPS, 128, G * NCH, 128), ml_dtypes.bfloat16)
    np.put_along_axis(
        ohv, np.clip(dcol, 0, 127)[..., None].astype(np.int64),
        np.where(dcol >= 0, vcol, ml_dtypes.bfloat16(0))[..., None], axis=-1,
    )

    x_pad = np.zeros((int(QS[-1]), DP), ml_dtypes.bfloat16)
    x_pad[:N, :D] = x.astype(ml_dtypes.bfloat16)

    in_maps = []
    for k in range(CORES):
        in_maps.append(
            {
                "xq": x_pad,
                "w": np.ascontiguousarray(weight).astype(ml_dtypes.bfloat16),
                "gidx": np.ascontiguousarray(gidx[k]),
                "ohv": ohv[k],
            }
        )
    return in_maps, tuple(int(c) for c in Cq), perm


# ------------------------------------------------------------- bass program
def _build(Cq):
    import concourse.bacc as bacc
    import concourse.mybir as mybir
    import concourse.tile as tile

    f32 = mybir.dt.float32
    bf16 = mybir.dt.bfloat16
    i16 = mybir.dt.int16
    SLq = [c * 128 for c in Cq]
    NCH = sum(Cq)
    qchunkoff = [0]
    for c in Cq[:-1]:
        qchunkoff.append(qchunkoff[-1] + c)
    CALLE = [G * sl for sl in SLq]
    off16 = [0]
    for c in CALLE:
        off16.append(off16[-1] + c // 16)
    TOT16 = off16[-1]
    GR = G * 128                                  # rows per supergroup

    nc = bacc.Bacc(
        "TRN2",
        target_bir_lowering=False,
        debug=False,
        num_devices=CORES,
        num_swdge_queues=4,
    )
    NX = int(QS[-1])
    x_d = nc.dram_tensor("xq", [NX, DP], bf16, kind="ExternalInput")
    w_d = nc.dram_tensor("w", [D, D], bf16, kind="ExternalInput")
    gidx_d = nc.dram_tensor("gidx", [NGROUPS, 128, TOT16], i16, kind="ExternalInput")
    ohv_d = nc.dram_tensor(
        "ohv", [NGROUPS, 128, G * NCH, 128], bf16, kind="ExternalInput"
    )
    outT_d = nc.dram_tensor("outT", [D, RPC], f32, kind="ExternalOutput")

    with tile.TileContext(nc) as tc:
        with (
            tc.tile_pool(name="const", bufs=1) as cpool,
            tc.tile_pool(name="io", bufs=3) as iopool,
            tc.tile_pool(name="oh", bufs=2) as ohpool,
            tc.tile_pool(name="agg", bufs=2) as aggpool,
            tc.tile_pool(name="outsb", bufs=2) as opool,
            tc.tile_pool(name="pa", bufs=4, space="PSUM") as papool,
            tc.tile_pool(name="po", bufs=2, space="PSUM") as popool,
        ):
            w_sb = cpool.tile([D, D], bf16, name="w_sb")
            nc.scalar.dma_start(out=w_sb[:], in_=w_d[:])

            # persistent triple-buffered msgs tiles (gather fills every slot;
            # idx pads gather row 0, so contents are always finite; padded
            # slots have all-zero one-hot columns so they contribute nothing).
            # 3 buffers so group g's gathers only wait on the matmuls of
            # group g-3 -- two full group-periods of slack
            NB = 3
            msgs_t = [
                [
                    cpool.tile([128, G, Cq[q], DP], bf16, name=f"msgs{bi}_{q}")
                    for q in range(Q)
                ]
                for bi in range(NB)
            ]

            def emit_block(b, oh_t, aggT_g, rhs_fn):
                # aggT[64, 128 rows] += msgs_chunk[:, :64].T @ ohv_chunk
                lb = b % G
                pa = papool.tile([D, 128], f32, tag="pa", name=f"pa{b}")
                i = 0
                for q in range(Q):
                    for c in range(Cq[q]):
                        nc.tensor.matmul(
                            pa[:],
                            rhs_fn(q, c),
                            oh_t[:, lb * NCH + qchunkoff[q] + c, :],
                            start=(i == 0),
                            stop=(i == NCH - 1),
                        )
                        i += 1
                nc.scalar.copy(aggT_g[:, lb * 128 : (lb + 1) * 128], pa[:])

            def emit_group_out(g, aggT_g):
                # outT[64, GR] = W.T @ aggT slice; 2 matmuls (PSUM bank <=512)
                ot = opool.tile([D, GR], f32, tag="ot", name=f"ot{g}")
                for j, (o0, nn) in enumerate(((0, 512), (512, GR - 512))):
                    po = popool.tile([D, nn], f32, tag=f"po{j}", name=f"po{g}_{j}")
                    nc.tensor.matmul(
                        po[:],
                        w_sb[:],
                        aggT_g[:, o0 : o0 + nn],
                        start=True,
                        stop=True,
                    )
                    nc.scalar.copy(ot[:, o0 : o0 + nn], po[:])
                # scalar queue: keeps the sync queue a pure idx stream, so
                # the gathers' idx waits never count a slow DMA on a shared
                # DMAHW sem lane
                nc.scalar.dma_start(
                    out=outT_d[:, g * GR : (g + 1) * GR], in_=ot[:]
                )

            def load_io(g):
                # idx on the sync queue; the one-hot in 7 per-block pieces on
                # the (otherwise idle) Activation queue.  One big 1.95MB oh
                # DMA stalls the gathers ~18us/group: the Tile DMA-completion
                # sem lanes (DMAHW0-7) are shared round-robin across HWDGE
                # DMAs, so the gathers' idx wait transitively counts the oh
                # completion.  Small pieces complete in ~1us each, so the
                # false coupling costs nothing.
                idx_t = iopool.tile([128, TOT16], i16, tag="idx", name=f"idx{g}")
                oh_t = ohpool.tile(
                    [128, G * NCH, 128], bf16, tag="oh", name=f"oh{g}"
                )
                nc.sync.dma_start(out=idx_t[:], in_=gidx_d[g])
                for lb in range(G):
                    nc.scalar.dma_start(
                        out=oh_t[:, lb * NCH : (lb + 1) * NCH, :],
                        in_=ohv_d[g, :, lb * NCH : (lb + 1) * NCH, :],
                    )
                return idx_t, oh_t

            io_next = load_io(0)
            for g in range(NGROUPS):
                idx_t, oh_t = io_next
                if g + 1 < NGROUPS:
                    io_next = load_io(g + 1)
                aggT_g = aggpool.tile([D, GR], bf16, tag="aggT", name=f"aggT{g}")

                if g < NGROUPS - 1:
                    msgs = msgs_t[g % NB]
                    for q in range(Q):
                        m = msgs[q]
                        nc.gpsimd.dma_gather(
                            m[:].rearrange("p g c d -> p (g c) d"),
                            x_d[int(QS[q]) : int(QS[q + 1]), :],
                            idx_t[:, off16[q] : off16[q + 1]],
                            CALLE[q],
                            CALLE[q],
                            DP,
                            # single_packet=True needs the whole call inside
                            # the 1024-desc SWDGE ring -> crash on big calls
                            single_packet=False,
                            # round-robin the 4 SWDGE queues: each runs on its
                            # own Q7 core pair, so desc-gen parallelizes 4x
                            queue_num=(g + q) % 4,
                        )
                    for lb in range(G):
                        b = g * G + lb
                        emit_block(
                            b, oh_t, aggT_g,
                            lambda q, c, _m=msgs, _lb=lb: _m[q][:, _lb, c, :D],
                        )
                    emit_group_out(g, aggT_g)
                else:
                    # taper the final supergroup: per-block calls into
                    # dedicated ping-pong tiles so each block's compute
                    # overlaps the next block's gather, and the kernel tail
                    # is one block rather than a whole supergroup
                    for lb in range(G):
                        # alternate tiles 1/2 (tile 0 is still owned by the
                        # just-finished group 12); distinct tiles let block
                        # lb+1's gather overlap block lb's matmuls even with
                        # whole-tile dependency tracking
                        msgs = msgs_t[1 + lb % 2]
                        for q in range(Q):
                            nc.gpsimd.dma_gather(
                                msgs[q][:, lb, :, :],
                                x_d[int(QS[q]) : int(QS[q + 1]), :],
                                idx_t[
                                    :,
                                    off16[q]
                                    + lb * (SLq[q] // 16) : off16[q]
                                    + (lb + 1) * (SLq[q] // 16),
                                ],
                                SLq[q],
                                SLq[q],
                                DP,
                                single_packet=False,
                                # queue 0 carried group 12's big call and
                                # drains last; start the taper on queue 1
                                queue_num=(1 + lb + q) % 4,
                            )
                        b = g * G + lb
                        emit_block(
                            b, oh_t, aggT_g,
                            lambda q, c, _m=msgs, _lb=lb: _m[q][:, _lb, c, :D],
                        )
                    emit_group_out(g, aggT_g)

    nc.compile()
    return nc


# ----------------------------------------------------------------- kernel()
def _ensure_ntff_hook():
    """Provide antenv.axon_hooks (absent in this image) so that
    run_bass_kernel_spmd's BASS_TRACE path can register the axon NTFF
    profiler instead of crashing on import."""
    try:
        import antenv.axon_hooks  # noqa: F401

        return
    except ImportError:
        pass
    import types

    import antenv

    mod = types.ModuleType("antenv.axon_hooks")
    holder = {"hook": None}
    mod.set_axon_ntff_profile_hook = lambda h: holder.__setitem__("hook", h)
    mod.get_axon_ntff_profile_hook = lambda: holder["hook"]
    sys.modules["antenv.axon_hooks"] = mod
    antenv.axon_hooks = mod
    try:
        from trn_agent_boot.trn_boot import _ntff_profile_via_ctypes

        mod.set_axon_ntff_profile_hook(
            _ntff_profile_via_ctypes("/opt/axon/libaxon_pjrt.so")
        )
    except Exception:
        pass


def kernel(x, weight, edge_vals, edge_row, edge_col):
    global LAST_EXEC_TIME_NS
    from concourse.bass_utils import run_bass_kernel_spmd

    if os.environ.get("BASS_TRACE"):
        _ensure_ntff_hook()

    in_maps, Cq, perm = _prep(x, weight, edge_vals, edge_row, edge_col)
    if Cq not in _CACHE:
        _CACHE[Cq] = _build(Cq)
    nc = _CACHE[Cq]

    res = run_bass_kernel_spmd(nc, in_maps, list(range(CORES)))
    LAST_EXEC_TIME_NS = res.exec_time_ns

    out = np.empty((CORES * RPC, D), np.float32)
    for k in range(CORES):
        out[perm[k]] = res.results[k]["outT"].T
    return np.ascontiguousarray(out[:N])


# revision 19
# speedup vs baseline: 1.0343x; 1.0105x over previous
"""GCNConv (COO SpMM aggregation + dense GEMM) on 8 Trainium2 NeuronCores.

  msgs = edge_vals[:, None] * x[edge_col]          # [E, 64] gather+scale
  agg  = segment_sum(msgs, edge_row, N)            # [N, 64] scatter-add
  out  = agg @ weight                              # [N, 64] GEMM

Sharding: destination-node sharding (each core owns a contiguous row slab and
all edges targeting it) -> zero collectives.

The throughput limit is SWDGE descriptor generation for the per-edge row
gather (~9.6 ns/descriptor on one Q7 pair).  The kernel splits the gather
calls across all 4 SWDGE queues (each queue runs on its own Q7 core pair),
parallelizing descriptor generation 4x.  Everything else is arranged to hide
under that wall:
  - x is stored bf16, feature-padded to 128 cols so each row is a 256B gather
    element; the gather output is directly the TensorE stationary operand.
  - the edge_vals scaling AND the destination one-hot are merged into a
    HOST-BUILT val-weighted bf16 one-hot (ohv[slot, r] = val if dest==r else
    0; padded slots all-zero), streamed from HBM.  The Vector engine does no
    per-edge work at all.
  - TensorE per 128-edge chunk (bf16): psum_aggT[64, 128 rows] +=
    msgs[:, :64].T @ ohv  -- transposed aggregates directly, no PE transpose.
  - Activation engine copies psum_aggT -> aggT (bf16), and the per-supergroup
    W GEMM outT[64, 896] = W.T @ aggT runs as 2 matmuls + ACT copy + one
    contiguous output DMA; host scatters rows back.

Host-side prep minimizes padded gather slots:
  - x is split into 4 unequal quarters (int16 gather indices), sized so each
    (block, quarter) edge-group mean sits well below a multiple of 128.
  - each core's 12544 rows are bin-packed into 98 blocks of 128 rows,
    balancing all 4 per-quarter degree sums; the row permutation is undone
    on the host at the end.
"""

import os
import sys

import numpy as np

if "/opt/trn_rl_repo" not in sys.path:
    sys.path.insert(0, "/opt/trn_rl_repo")

import ml_dtypes

# ---------------------------------------------------------------- constants
N = 100000
E = 1600000
D = 64
DP = 128             # padded feature count (256B bf16 gather elements)
CORES = 8
RPC = 12544          # rows per core (8*12544 = 100352 >= N)
BLOCKS = RPC // 128  # 98 dest blocks per core
Q = 4
QS = np.array([0, 30134, 54243, 78352, 100352], dtype=np.int64)  # quarter bounds
CAPQ = np.array([640, 512, 512, 512], dtype=np.int64)  # packing targets
G = 7                # dest blocks per gather super-group (98 = 14*7)
NGROUPS = BLOCKS // G

LAST_EXEC_TIME_NS = None
_CACHE = {}


def _pack_rows(deg):
    """Assign RPC rows (deg: [RPC, 4] per-quarter degrees) to BLOCKS blocks
    of 128, balancing all 4 quarter sums against the CAPQ targets.  Greedy
    rounds (one row per block per round) + peak-shaving swap repair.
    Returns perm_local[pos] = row, where pos = block*128 + slot."""
    order = np.argsort(-deg.sum(1), kind="stable")
    cur = np.zeros((BLOCKS, Q), np.float64)
    capf = CAPQ.astype(np.float64)
    blk_of = np.empty(RPC, np.int64)
    for rnd in range(128):
        batch = order[rnd * BLOCKS : (rnd + 1) * BLOCKS]
        bscore = (deg[batch] / capf).max(1)
        bo = batch[np.argsort(-bscore, kind="stable")]
        load = (cur / capf).max(1)
        blko = np.argsort(load, kind="stable")
        cur[blko] += deg[bo]
        blk_of[bo] = blko
    # repair: swap the heaviest row (in the hottest quarter) of the hottest
    # block with a light row of the coolest block
    loadi = np.zeros((BLOCKS, Q), np.int64)
    np.add.at(loadi, blk_of, deg)
    rows_in = [list(np.where(blk_of == b)[0]) for b in range(BLOCKS)]
    for _ in range(4000):
        nl = loadi / capf
        b, q = np.unravel_index(np.argmax(nl), nl.shape)
        b, q = int(b), int(q)
        if nl[b, q] <= 1.0:
            break
        cand = max(rows_in[b], key=lambda r: deg[r, q])
        tgt = int(np.argmin(nl[:, q] + (np.arange(BLOCKS) == b) * 10))
        cand2 = min(rows_in[tgt], key=lambda r: deg[r, q])
        loadi[b] += deg[cand2] - deg[cand]
        loadi[tgt] += deg[cand] - deg[cand2]
        rows_in[b].remove(cand)
        rows_in[b].append(cand2)
        rows_in[tgt].remove(cand2)
        rows_in[tgt].append(cand)
    perm_local = np.empty(RPC, np.int64)
    for b in range(BLOCKS):
        for j, r in enumerate(rows_in[b]):
            perm_local[b * 128 + j] = r
    return perm_local


# ---------------------------------------------------------------- host prep
def _prep(x, weight, edge_vals, edge_row, edge_col):
    e_row = np.asarray(edge_row, dtype=np.int64)
    e_col = np.asarray(edge_col, dtype=np.int64)
    ev = np.asarray(edge_vals, dtype=np.float32)
    x = np.asarray(x, dtype=np.float32)
    weight = np.asarray(weight, dtype=np.float32)
    ne = e_row.shape[0]
    NPAD = CORES * RPC

    qq = np.searchsorted(QS, e_col, side="right") - 1
    lidx = (e_col - QS[qq]).astype(np.int16)

    # per-row per-quarter degrees -> per-core packing permutation
    deg_flat = np.bincount(e_row * Q + qq, minlength=NPAD * Q).reshape(NPAD, Q)
    perm = np.empty((CORES, RPC), np.int64)      # perm[k, pos] = global row
    pos_of_row = np.empty(NPAD, np.int64)        # core-local position
    for k in range(CORES):
        pl = _pack_rows(deg_flat[k * RPC : (k + 1) * RPC])
        perm[k] = k * RPC + pl
        pos_of_row[perm[k]] = np.arange(RPC)

    core = e_row // RPC
    pos = pos_of_row[e_row]
    blk = pos // 128
    dest = (pos % 128).astype(np.int16)

    # group counts -> per-quarter chunk counts (global static)
    gkey = (core * BLOCKS + blk) * Q + qq
    counts = np.bincount(gkey, minlength=CORES * BLOCKS * Q)
    cmax = counts.reshape(CORES * BLOCKS, Q).max(axis=0)
    Cq = np.maximum(1, -(-cmax // 128))          # [Q] chunks per group
    SLq = Cq * 128
    SLOTSB = int(SLq.sum())                      # slots per block
    NCH = int(Cq.sum())                          # chunk-columns per block
    qslotoff = np.concatenate([[0], np.cumsum(SLq)[:-1]])

    order = np.argsort(gkey, kind="stable")
    NGK = CORES * BLOCKS * Q
    starts = np.zeros(NGK, np.int64)
    starts[1:] = np.cumsum(counts)[:-1]
    gsort = gkey[order]
    rank = np.arange(ne, dtype=np.int64) - starts[gsort]
    cb = gsort // Q
    qs = gsort % Q
    slot = cb * SLOTSB + qslotoff[qs] + rank

    NSLOT = CORES * BLOCKS * SLOTSB
    idx_flat = np.zeros(NSLOT, np.int16)          # pad gathers row 0
    dst_flat = np.full(NSLOT, -1, np.int16)       # pad -> all-zero onehot col
    val_flat = np.zeros(NSLOT, np.float32)        # pad scales to 0
    idx_flat[slot] = lidx[order]
    dst_flat[slot] = dest[order]
    val_flat[slot] = ev[order]

    slots = idx_flat.reshape(CORES, NGROUPS, G, SLOTSB)
    dsts = dst_flat.reshape(CORES, NGROUPS, G, SLOTSB)
    vals = val_flat.reshape(CORES, NGROUPS, G, SLOTSB)

    # gather idx per call (g, q): [G*SLq] block-major; wrap to [128, ./16]
    gi_parts = []
    for q in range(Q):
        arr = slots[:, :, :, qslotoff[q] : qslotoff[q] + SLq[q]]
        arr = np.ascontiguousarray(arr).reshape(CORES, NGROUPS, G * int(SLq[q]))
        w16 = arr.reshape(CORES, NGROUPS, -1, 16)
        w16 = np.moveaxis(w16, 3, 2)             # [C, NGR, 16, CALLE/16]
        gi_parts.append(np.tile(w16, (1, 1, 8, 1)))
    gidx = np.ascontiguousarray(np.concatenate(gi_parts, axis=3))

    # host-built val-weighted one-hot, chunk-column layout
    # [C, NGR, 128, G*NCH, 128]: column (lb, q, c) = lb*NCH + qchunkoff[q] + c,
    # ohv[., ., p, col, r] = val[slot] if dest[slot] == r else 0
    def to_cols(a):
        parts = []
        for q in range(Q):
            seg = a[:, :, :, qslotoff[q] : qslotoff[q] + SLq[q]]
            parts.append(
                np.ascontiguousarray(seg).reshape(
                    CORES, NGROUPS, G, int(Cq[q]), 128
                )
            )
        cols = np.concatenate(parts, axis=3)      # [C, NGR, G, NCH, 128]
        cols = cols.reshape(CORES, NGROUPS, G * NCH, 128)
        return np.ascontiguousarray(np.moveaxis(cols, 3, 2))

    dcol = to_cols(dsts)                          # [C, NGR, 128, G*NCH] int16
    vcol = to_cols(vals).astype(ml_dtypes.bfloat16)
    ohv = np.zeros((CORES, NGROUPS, 128, G * NCH, 128), ml_dtypes.bfloat16)
    np.put_along_axis(
        ohv, np.clip(dcol, 0, 127)[..., None].astype(np.int64),
        np.where(dcol >= 0, vcol, ml_dtypes.bfloat16(0))[..., None], axis=-1,
    )

    x_pad = np.zeros((int(QS[-1]), DP), ml_dtypes.bfloat16)
    x_pad[:N, :D] = x.astype(ml_dtypes.bfloat16)

    in_maps = []
    for k in range(CORES):
        in_maps.append(
            {
                "xq": x_pad,
                "w": np.ascontiguousarray(weight).astype(ml_dtypes.bfloat16),
                "gidx": np.ascontiguousarray(gidx[k]),
                "ohv": ohv[k],
            }
        )
    return in_maps, tuple(int(c) for c in Cq), perm


# ------------------------------------------------------------- bass program
def _build(Cq):
    import concourse.bacc as bacc
    import concourse.mybir as mybir
    import concourse.tile as tile

    f32 = mybir.dt.float32
    bf16 = mybir.dt.bfloat16
    i16 = mybir.dt.int16
    SLq = [c * 128 for c in Cq]
    NCH = sum(Cq)
    qchunkoff = [0]
    for c in Cq[:-1]:
        qchunkoff.append(qchunkoff[-1] + c)
    CALLE = [G * sl for sl in SLq]
    off16 = [0]
    for c in CALLE:
        off16.append(off16[-1] + c // 16)
    TOT16 = off16[-1]
    GR = G * 128                                  # rows per supergroup

    nc = bacc.Bacc(
        "TRN2",
        target_bir_lowering=False,
        debug=False,
        num_devices=CORES,
        num_swdge_queues=4,
    )
    NX = int(QS[-1])
    x_d = nc.dram_tensor("xq", [NX, DP], bf16, kind="ExternalInput")
    w_d = nc.dram_tensor("w", [D, D], bf16, kind="ExternalInput")
    gidx_d = nc.dram_tensor("gidx", [NGROUPS, 128, TOT16], i16, kind="ExternalInput")
    ohv_d = nc.dram_tensor(
        "ohv", [NGROUPS, 128, G * NCH, 128], bf16, kind="ExternalInput"
    )
    outT_d = nc.dram_tensor("outT", [D, RPC], f32, kind="ExternalOutput")

    with tile.TileContext(nc) as tc:
        with (
            tc.tile_pool(name="const", bufs=1) as cpool,
            tc.tile_pool(name="io", bufs=3) as iopool,
            tc.tile_pool(name="oh", bufs=2) as ohpool,
            tc.tile_pool(name="agg", bufs=2) as aggpool,
            tc.tile_pool(name="outsb", bufs=2) as opool,
            tc.tile_pool(name="pa", bufs=4, space="PSUM") as papool,
            tc.tile_pool(name="po", bufs=2, space="PSUM") as popool,
        ):
            w_sb = cpool.tile([D, D], bf16, name="w_sb")
            nc.scalar.dma_start(out=w_sb[:], in_=w_d[:])

            # persistent triple-buffered msgs tiles (gather fills every slot;
            # idx pads gather row 0, so contents are always finite; padded
            # slots have all-zero one-hot columns so they contribute nothing).
            # 3 buffers so group g's gathers only wait on the matmuls of
            # group g-3 -- two full group-periods of slack
            NB = 3
            msgs_t = [
                [
                    cpool.tile([128, G, Cq[q], DP], bf16, name=f"msgs{bi}_{q}")
                    for q in range(Q)
                ]
                for bi in range(NB)
            ]

            def emit_block(b, oh_t, aggT_g, rhs_fn):
                # aggT[64, 128 rows] += msgs_chunk[:, :64].T @ ohv_chunk
                lb = b % G
                pa = papool.tile([D, 128], f32, tag="pa", name=f"pa{b}")
                i = 0
                for q in range(Q):
                    for c in range(Cq[q]):
                        nc.tensor.matmul(
                            pa[:],
                            rhs_fn(q, c),
                            oh_t[:, lb * NCH + qchunkoff[q] + c, :],
                            start=(i == 0),
                            stop=(i == NCH - 1),
                        )
                        i += 1
                nc.scalar.copy(aggT_g[:, lb * 128 : (lb + 1) * 128], pa[:])

            def emit_group_out(g, aggT_g):
                # outT[64, GR] = W.T @ aggT slice; 2 matmuls (PSUM bank <=512)
                ot = opool.tile([D, GR], f32, tag="ot", name=f"ot{g}")
                for j, (o0, nn) in enumerate(((0, 512), (512, GR - 512))):
                    po = popool.tile([D, nn], f32, tag=f"po{j}", name=f"po{g}_{j}")
                    nc.tensor.matmul(
                        po[:],
                        w_sb[:],
                        aggT_g[:, o0 : o0 + nn],
                        start=True,
                        stop=True,
                    )
                    nc.scalar.copy(ot[:, o0 : o0 + nn], po[:])
                # scalar queue: keeps the sync queue a pure idx stream, so
                # the gathers' idx waits never count a slow DMA on a shared
                # DMAHW sem lane
                nc.scalar.dma_start(
                    out=outT_d[:, g * GR : (g + 1) * GR], in_=ot[:]
                )

            def load_io(g):
                # idx on the sync queue; the one-hot in 7 per-block pieces on
                # the (otherwise idle) Activation queue.  One big 1.95MB oh
                # DMA stalls the gathers ~18us/group: the Tile DMA-completion
                # sem lanes (DMAHW0-7) are shared round-robin across HWDGE
                # DMAs, so the gathers' idx wait transitively counts the oh
                # completion.  Small pieces complete in ~1us each, so the
                # false coupling costs nothing.
                idx_t = iopool.tile([128, TOT16], i16, tag="idx", name=f"idx{g}")
                oh_t = ohpool.tile(
                    [128, G * NCH, 128], bf16, tag="oh", name=f"oh{g}"
                )
                nc.sync.dma_start(out=idx_t[:], in_=gidx_d[g])
                for lb in range(G):
                    nc.scalar.dma_start(
                        out=oh_t[:, lb * NCH : (lb + 1) * NCH, :],
                        in_=ohv_d[g, :, lb * NCH : (lb + 1) * NCH, :],
                    )
                return idx_t, oh_t

            io_next = load_io(0)
            for g in range(NGROUPS):
                idx_t, oh_t = io_next
                if g + 1 < NGROUPS:
                    io_next = load_io(g + 1)
                aggT_g = aggpool.tile([D, GR], bf16, tag="aggT", name=f"aggT{g}")

                if g < NGROUPS - 1:
                    msgs = msgs_t[g % NB]
                    for q in range(Q):
                        m = msgs[q]
                        nc.gpsimd.dma_gather(
                            m[:].rearrange("p g c d -> p (g c) d"),
                            x_d[int(QS[q]) : int(QS[q + 1]), :],
                            idx_t[:, off16[q] : off16[q + 1]],
                            CALLE[q],
                            CALLE[q],
                            DP,
                            # single_packet=True needs the whole call inside
                            # the 1024-desc SWDGE ring -> crash on big calls
                            single_packet=False,
                            # round-robin the 4 SWDGE queues: each runs on its
                            # own Q7 core pair, so desc-gen parallelizes 4x
                            queue_num=(g + q) % 4,
                        )
                    for lb in range(G):
                        b = g * G + lb
                        emit_block(
                            b, oh_t, aggT_g,
                            lambda q, c, _m=msgs, _lb=lb: _m[q][:, _lb, c, :D],
                        )
                    emit_group_out(g, aggT_g)
                else:
                    # taper the final supergroup: per-block calls into
                    # dedicated ping-pong tiles so each block's compute
                    # overlaps the next block's gather, and the kernel tail
                    # is one block rather than a whole supergroup
                    for lb in range(G):
                        # alternate tiles 1/2 (tile 0 is still owned by the
                        # just-finished group 12); distinct tiles let block
                        # lb+1's gather overlap block lb's matmuls even with
                        # whole-tile dependency tracking
                        msgs = msgs_t[1 + lb % 2]
                        for q in range(Q):
                            nc.gpsimd.dma_gather(
                                msgs[q][:, lb, :, :],
                                x_d[int(QS[q]) : int(QS[q + 1]), :],
                                idx_t[
                                    :,
                                    off16[q]
                                    + lb * (SLq[q] // 16) : off16[q]
                                    + (lb + 1) * (SLq[q] // 16),
                                ],
                                SLq[q],
                                SLq[q],
                                DP,
                                single_packet=False,
                                queue_num=(lb + q) % 4,
                            )
                        b = g * G + lb
                        emit_block(
                            b, oh_t, aggT_g,
                            lambda q, c, _m=msgs, _lb=lb: _m[q][:, _lb, c, :D],
                        )
                    emit_group_out(g, aggT_g)

    nc.compile()
    return nc


# ----------------------------------------------------------------- kernel()
def _ensure_ntff_hook():
    """Provide antenv.axon_hooks (absent in this image) so that
    run_bass_kernel_spmd's BASS_TRACE path can register the axon NTFF
    profiler instead of crashing on import."""
    try:
        import antenv.axon_hooks  # noqa: F401

        return
    except ImportError:
        pass
    import types

    import antenv

    mod = types.ModuleType("antenv.axon_hooks")
    holder = {"hook": None}
    mod.set_axon_ntff_profile_hook = lambda h: holder.__setitem__("hook", h)
    mod.get_axon_ntff_profile_hook = lambda: holder["hook"]
    sys.modules["antenv.axon_hooks"] = mod
    antenv.axon_hooks = mod
    try:
        from trn_agent_boot.trn_boot import _ntff_profile_via_ctypes

        mod.set_axon_ntff_profile_hook(
            _ntff_profile_via_ctypes("/opt/axon/libaxon_pjrt.so")
        )
    except Exception:
        pass


def kernel(x, weight, edge_vals, edge_row, edge_col):
    global LAST_EXEC_TIME_NS
    from concourse.bass_utils import run_bass_kernel_spmd

    if os.environ.get("BASS_TRACE"):
        _ensure_ntff_hook()

    in_maps, Cq, perm = _prep(x, weight, edge_vals, edge_row, edge_col)
    if Cq not in _CACHE:
        _CACHE[Cq] = _build(Cq)
    nc = _CACHE[Cq]

    res = run_bass_kernel_spmd(nc, in_maps, list(range(CORES)))
    LAST_EXEC_TIME_NS = res.exec_time_ns

    out = np.empty((CORES * RPC, D), np.float32)
    for k in range(CORES):
        out[perm[k]] = res.results[k]["outT"].T
    return np.ascontiguousarray(out[:N])


# revision 20
# speedup vs baseline: 1.0859x; 1.0499x over previous
"""GCNConv (COO SpMM aggregation + dense GEMM) on 8 Trainium2 NeuronCores.

  msgs = edge_vals[:, None] * x[edge_col]          # [E, 64] gather+scale
  agg  = segment_sum(msgs, edge_row, N)            # [N, 64] scatter-add
  out  = agg @ weight                              # [N, 64] GEMM

Sharding: destination-node sharding (each core owns a contiguous row slab and
all edges targeting it) -> zero collectives.

The throughput limit is SWDGE descriptor generation for the per-edge row
gather (~9.6 ns/descriptor on one Q7 pair).  The kernel splits the gather
calls across all 4 SWDGE queues (each queue runs on its own Q7 core pair),
parallelizing descriptor generation 4x.  Everything else is arranged to hide
under that wall:
  - x is stored bf16, feature-padded to 128 cols so each row is a 256B gather
    element; the gather output is directly the TensorE stationary operand.
  - the edge_vals scaling AND the destination one-hot are merged into a
    HOST-BUILT val-weighted bf16 one-hot (ohv[slot, r] = val if dest==r else
    0; padded slots all-zero), streamed from HBM.  The Vector engine does no
    per-edge work at all.
  - TensorE per 128-edge chunk (bf16): psum_aggT[64, 128 rows] +=
    msgs[:, :64].T @ ohv  -- transposed aggregates directly, no PE transpose.
  - Activation engine copies psum_aggT -> aggT (bf16), and the per-supergroup
    W GEMM outT[64, 896] = W.T @ aggT runs as 2 matmuls + ACT copy + one
    contiguous output DMA; host scatters rows back.

Host-side prep minimizes padded gather slots:
  - x is split into 4 unequal quarters (int16 gather indices), sized so each
    (block, quarter) edge-group mean sits well below a multiple of 128.
  - each core's 12544 rows are bin-packed into 98 blocks of 128 rows,
    balancing all 4 per-quarter degree sums; the row permutation is undone
    on the host at the end.
"""

import os
import sys

import numpy as np

if "/opt/trn_rl_repo" not in sys.path:
    sys.path.insert(0, "/opt/trn_rl_repo")

import ml_dtypes

# ---------------------------------------------------------------- constants
N = 100000
E = 1600000
D = 64
DP = 128             # padded feature count (256B bf16 gather elements)
CORES = 8
RPC = 12544          # rows per core (8*12544 = 100352 >= N)
BLOCKS = RPC // 128  # 98 dest blocks per core
Q = 4
QS = np.array([0, 30134, 54243, 78352, 100352], dtype=np.int64)  # quarter bounds
CAPQ = np.array([640, 512, 512, 512], dtype=np.int64)  # packing targets
G = 7                # dest blocks per gather super-group (98 = 14*7)
NGROUPS = BLOCKS // G

LAST_EXEC_TIME_NS = None
_CACHE = {}


def _pack_rows(deg):
    """Assign RPC rows (deg: [RPC, 4] per-quarter degrees) to BLOCKS blocks
    of 128, balancing all 4 quarter sums against the CAPQ targets.  Greedy
    rounds (one row per block per round) + peak-shaving swap repair.
    Returns perm_local[pos] = row, where pos = block*128 + slot."""
    order = np.argsort(-deg.sum(1), kind="stable")
    cur = np.zeros((BLOCKS, Q), np.float64)
    capf = CAPQ.astype(np.float64)
    blk_of = np.empty(RPC, np.int64)
    for rnd in range(128):
        batch = order[rnd * BLOCKS : (rnd + 1) * BLOCKS]
        bscore = (deg[batch] / capf).max(1)
        bo = batch[np.argsort(-bscore, kind="stable")]
        load = (cur / capf).max(1)
        blko = np.argsort(load, kind="stable")
        cur[blko] += deg[bo]
        blk_of[bo] = blko
    # repair: swap the heaviest row (in the hottest quarter) of the hottest
    # block with a light row of the coolest block
    loadi = np.zeros((BLOCKS, Q), np.int64)
    np.add.at(loadi, blk_of, deg)
    rows_in = [list(np.where(blk_of == b)[0]) for b in range(BLOCKS)]
    for _ in range(4000):
        nl = loadi / capf
        b, q = np.unravel_index(np.argmax(nl), nl.shape)
        b, q = int(b), int(q)
        if nl[b, q] <= 1.0:
            break
        cand = max(rows_in[b], key=lambda r: deg[r, q])
        tgt = int(np.argmin(nl[:, q] + (np.arange(BLOCKS) == b) * 10))
        cand2 = min(rows_in[tgt], key=lambda r: deg[r, q])
        loadi[b] += deg[cand2] - deg[cand]
        loadi[tgt] += deg[cand] - deg[cand2]
        rows_in[b].remove(cand)
        rows_in[b].append(cand2)
        rows_in[tgt].remove(cand2)
        rows_in[tgt].append(cand)
    perm_local = np.empty(RPC, np.int64)
    for b in range(BLOCKS):
        for j, r in enumerate(rows_in[b]):
            perm_local[b * 128 + j] = r
    return perm_local


# ---------------------------------------------------------------- host prep
def _prep(x, weight, edge_vals, edge_row, edge_col):
    e_row = np.asarray(edge_row, dtype=np.int64)
    e_col = np.asarray(edge_col, dtype=np.int64)
    ev = np.asarray(edge_vals, dtype=np.float32)
    x = np.asarray(x, dtype=np.float32)
    weight = np.asarray(weight, dtype=np.float32)
    ne = e_row.shape[0]
    NPAD = CORES * RPC

    qq = np.searchsorted(QS, e_col, side="right") - 1
    lidx = (e_col - QS[qq]).astype(np.int16)

    # per-row per-quarter degrees -> per-core packing permutation
    deg_flat = np.bincount(e_row * Q + qq, minlength=NPAD * Q).reshape(NPAD, Q)
    perm = np.empty((CORES, RPC), np.int64)      # perm[k, pos] = global row
    pos_of_row = np.empty(NPAD, np.int64)        # core-local position
    for k in range(CORES):
        pl = _pack_rows(deg_flat[k * RPC : (k + 1) * RPC])
        perm[k] = k * RPC + pl
        pos_of_row[perm[k]] = np.arange(RPC)

    core = e_row // RPC
    pos = pos_of_row[e_row]
    blk = pos // 128
    dest = (pos % 128).astype(np.int16)

    # group counts -> per-quarter chunk counts (global static)
    gkey = (core * BLOCKS + blk) * Q + qq
    counts = np.bincount(gkey, minlength=CORES * BLOCKS * Q)
    cmax = counts.reshape(CORES * BLOCKS, Q).max(axis=0)
    Cq = np.maximum(1, -(-cmax // 128))          # [Q] chunks per group
    SLq = Cq * 128
    SLOTSB = int(SLq.sum())                      # slots per block
    NCH = int(Cq.sum())                          # chunk-columns per block
    qslotoff = np.concatenate([[0], np.cumsum(SLq)[:-1]])

    order = np.argsort(gkey, kind="stable")
    NGK = CORES * BLOCKS * Q
    starts = np.zeros(NGK, np.int64)
    starts[1:] = np.cumsum(counts)[:-1]
    gsort = gkey[order]
    rank = np.arange(ne, dtype=np.int64) - starts[gsort]
    cb = gsort // Q
    qs = gsort % Q
    slot = cb * SLOTSB + qslotoff[qs] + rank

    NSLOT = CORES * BLOCKS * SLOTSB
    idx_flat = np.zeros(NSLOT, np.int16)          # pad gathers row 0
    dst_flat = np.full(NSLOT, -1, np.int16)       # pad -> all-zero onehot col
    val_flat = np.zeros(NSLOT, np.float32)        # pad scales to 0
    idx_flat[slot] = lidx[order]
    dst_flat[slot] = dest[order]
    val_flat[slot] = ev[order]

    slots = idx_flat.reshape(CORES, NGROUPS, G, SLOTSB)
    dsts = dst_flat.reshape(CORES, NGROUPS, G, SLOTSB)
    vals = val_flat.reshape(CORES, NGROUPS, G, SLOTSB)

    # gather idx per call (g, q): [G*SLq] block-major; wrap to [128, ./16]
    gi_parts = []
    for q in range(Q):
        arr = slots[:, :, :, qslotoff[q] : qslotoff[q] + SLq[q]]
        arr = np.ascontiguousarray(arr).reshape(CORES, NGROUPS, G * int(SLq[q]))
        w16 = arr.reshape(CORES, NGROUPS, -1, 16)
        w16 = np.moveaxis(w16, 3, 2)             # [C, NGR, 16, CALLE/16]
        gi_parts.append(np.tile(w16, (1, 1, 8, 1)))
    gidx = np.ascontiguousarray(np.concatenate(gi_parts, axis=3))

    # host-built val-weighted one-hot, chunk-column layout
    # [C, NGR, 128, G*NCH, 128]: column (lb, q, c) = lb*NCH + qchunkoff[q] + c,
    # ohv[., ., p, col, r] = val[slot] if dest[slot] == r else 0
    def to_cols(a):
        parts = []
        for q in range(Q):
            seg = a[:, :, :, qslotoff[q] : qslotoff[q] + SLq[q]]
            parts.append(
                np.ascontiguousarray(seg).reshape(
                    CORES, NGROUPS, G, int(Cq[q]), 128
                )
            )
        cols = np.concatenate(parts, axis=3)      # [C, NGR, G, NCH, 128]
        cols = cols.reshape(CORES, NGROUPS, G * NCH, 128)
        return np.ascontiguousarray(np.moveaxis(cols, 3, 2))

    dcol = to_cols(dsts)                          # [C, NGR, 128, G*NCH] int16
    vcol = to_cols(vals).astype(ml_dtypes.bfloat16)
    ohv = np.zeros((CORES, NGROUPS, 128, G * NCH, 128), ml_dtypes.bfloat16)
    np.put_along_axis(
        ohv, np.clip(dcol, 0, 127)[..., None].astype(np.int64),
        np.where(dcol >= 0, vcol, ml_dtypes.bfloat16(0))[..., None], axis=-1,
    )

    x_pad = np.zeros((int(QS[-1]), DP), ml_dtypes.bfloat16)
    x_pad[:N, :D] = x.astype(ml_dtypes.bfloat16)

    in_maps = []
    for k in range(CORES):
        in_maps.append(
            {
                "xq": x_pad,
                "w": np.ascontiguousarray(weight).astype(ml_dtypes.bfloat16),
                "gidx": np.ascontiguousarray(gidx[k]),
                "ohv": ohv[k],
            }
        )
    return in_maps, tuple(int(c) for c in Cq), perm


# ------------------------------------------------------------- bass program
def _build(Cq):
    import concourse.bacc as bacc
    import concourse.mybir as mybir
    import concourse.tile as tile

    f32 = mybir.dt.float32
    bf16 = mybir.dt.bfloat16
    i16 = mybir.dt.int16
    SLq = [c * 128 for c in Cq]
    NCH = sum(Cq)
    qchunkoff = [0]
    for c in Cq[:-1]:
        qchunkoff.append(qchunkoff[-1] + c)
    CALLE = [G * sl for sl in SLq]
    off16 = [0]
    for c in CALLE:
        off16.append(off16[-1] + c // 16)
    TOT16 = off16[-1]
    GR = G * 128                                  # rows per supergroup

    nc = bacc.Bacc(
        "TRN2",
        target_bir_lowering=False,
        debug=False,
        num_devices=CORES,
        num_swdge_queues=4,
    )
    NX = int(QS[-1])
    x_d = nc.dram_tensor("xq", [NX, DP], bf16, kind="ExternalInput")
    w_d = nc.dram_tensor("w", [D, D], bf16, kind="ExternalInput")
    gidx_d = nc.dram_tensor("gidx", [NGROUPS, 128, TOT16], i16, kind="ExternalInput")
    ohv_d = nc.dram_tensor(
        "ohv", [NGROUPS, 128, G * NCH, 128], bf16, kind="ExternalInput"
    )
    outT_d = nc.dram_tensor("outT", [D, RPC], f32, kind="ExternalOutput")

    with tile.TileContext(nc) as tc:
        with (
            tc.tile_pool(name="const", bufs=1) as cpool,
            tc.tile_pool(name="io", bufs=3) as iopool,
            tc.tile_pool(name="oh", bufs=2) as ohpool,
            tc.tile_pool(name="agg", bufs=2) as aggpool,
            tc.tile_pool(name="outsb", bufs=2) as opool,
            tc.tile_pool(name="pa", bufs=4, space="PSUM") as papool,
            tc.tile_pool(name="po", bufs=2, space="PSUM") as popool,
        ):
            w_sb = cpool.tile([D, D], bf16, name="w_sb")
            nc.scalar.dma_start(out=w_sb[:], in_=w_d[:])

            # persistent triple-buffered msgs tiles (gather fills every slot;
            # idx pads gather row 0, so contents are always finite; padded
            # slots have all-zero one-hot columns so they contribute nothing).
            # 3 buffers so group g's gathers only wait on the matmuls of
            # group g-3 -- two full group-periods of slack
            NB = 3
            msgs_t = [
                [
                    cpool.tile([128, G, Cq[q], DP], bf16, name=f"msgs{bi}_{q}")
                    for q in range(Q)
                ]
                for bi in range(NB)
            ]

            def emit_block(b, oh_t, aggT_g, rhs_fn):
                # aggT[64, 128 rows] += msgs_chunk[:, :64].T @ ohv_chunk
                lb = b % G
                pa = papool.tile([D, 128], f32, tag="pa", name=f"pa{b}")
                i = 0
                for q in range(Q):
                    for c in range(Cq[q]):
                        nc.tensor.matmul(
                            pa[:],
                            rhs_fn(q, c),
                            oh_t[:, lb * NCH + qchunkoff[q] + c, :],
                            start=(i == 0),
                            stop=(i == NCH - 1),
                        )
                        i += 1
                nc.scalar.copy(aggT_g[:, lb * 128 : (lb + 1) * 128], pa[:])

            def emit_group_out(g, aggT_g):
                # outT[64, GR] = W.T @ aggT slice; 2 matmuls (PSUM bank <=512)
                ot = opool.tile([D, GR], f32, tag="ot", name=f"ot{g}")
                for j, (o0, nn) in enumerate(((0, 512), (512, GR - 512))):
                    po = popool.tile([D, nn], f32, tag=f"po{j}", name=f"po{g}_{j}")
                    nc.tensor.matmul(
                        po[:],
                        w_sb[:],
                        aggT_g[:, o0 : o0 + nn],
                        start=True,
                        stop=True,
                    )
                    nc.scalar.copy(ot[:, o0 : o0 + nn], po[:])
                # scalar queue: keeps the sync queue a pure idx stream, so
                # the gathers' idx waits never count a slow DMA on a shared
                # DMAHW sem lane
                nc.scalar.dma_start(
                    out=outT_d[:, g * GR : (g + 1) * GR], in_=ot[:]
                )

            def load_io(g):
                # idx on the sync queue; the one-hot in 7 per-block pieces on
                # the (otherwise idle) Activation queue.  One big 1.95MB oh
                # DMA stalls the gathers ~18us/group: the Tile DMA-completion
                # sem lanes (DMAHW0-7) are shared round-robin across HWDGE
                # DMAs, so the gathers' idx wait transitively counts the oh
                # completion.  Small pieces complete in ~1us each, so the
                # false coupling costs nothing.
                idx_t = iopool.tile([128, TOT16], i16, tag="idx", name=f"idx{g}")
                oh_t = ohpool.tile(
                    [128, G * NCH, 128], bf16, tag="oh", name=f"oh{g}"
                )
                nc.sync.dma_start(out=idx_t[:], in_=gidx_d[g])
                for lb in range(G):
                    nc.scalar.dma_start(
                        out=oh_t[:, lb * NCH : (lb + 1) * NCH, :],
                        in_=ohv_d[g, :, lb * NCH : (lb + 1) * NCH, :],
                    )
                return idx_t, oh_t

            io_next = load_io(0)
            for g in range(NGROUPS):
                idx_t, oh_t = io_next
                if g + 1 < NGROUPS:
                    io_next = load_io(g + 1)
                aggT_g = aggpool.tile([D, GR], bf16, tag="aggT", name=f"aggT{g}")

                if True:  # uniform groups; the old per-block taper cost more than it saved
                    msgs = msgs_t[g % NB]
                    for q in range(Q):
                        m = msgs[q]
                        nc.gpsimd.dma_gather(
                            m[:].rearrange("p g c d -> p (g c) d"),
                            x_d[int(QS[q]) : int(QS[q + 1]), :],
                            idx_t[:, off16[q] : off16[q + 1]],
                            CALLE[q],
                            CALLE[q],
                            DP,
                            # single_packet=True needs the whole call inside
                            # the 1024-desc SWDGE ring -> crash on big calls
                            single_packet=False,
                            # round-robin the 4 SWDGE queues: each runs on its
                            # own Q7 core pair, so desc-gen parallelizes 4x
                            queue_num=(g + q) % 4,
                        )
                    for lb in range(G):
                        b = g * G + lb
                        emit_block(
                            b, oh_t, aggT_g,
                            lambda q, c, _m=msgs, _lb=lb: _m[q][:, _lb, c, :D],
                        )
                    emit_group_out(g, aggT_g)

    nc.compile()
    return nc


# ----------------------------------------------------------------- kernel()
def _ensure_ntff_hook():
    """Provide antenv.axon_hooks (absent in this image) so that
    run_bass_kernel_spmd's BASS_TRACE path can register the axon NTFF
    profiler instead of crashing on import."""
    try:
        import antenv.axon_hooks  # noqa: F401

        return
    except ImportError:
        pass
    import types

    import antenv

    mod = types.ModuleType("antenv.axon_hooks")
    holder = {"hook": None}
    mod.set_axon_ntff_profile_hook = lambda h: holder.__setitem__("hook", h)
    mod.get_axon_ntff_profile_hook = lambda: holder["hook"]
    sys.modules["antenv.axon_hooks"] = mod
    antenv.axon_hooks = mod
    try:
        from trn_agent_boot.trn_boot import _ntff_profile_via_ctypes

        mod.set_axon_ntff_profile_hook(
            _ntff_profile_via_ctypes("/opt/axon/libaxon_pjrt.so")
        )
    except Exception:
        pass


def kernel(x, weight, edge_vals, edge_row, edge_col):
    global LAST_EXEC_TIME_NS
    from concourse.bass_utils import run_bass_kernel_spmd

    if os.environ.get("BASS_TRACE"):
        _ensure_ntff_hook()

    in_maps, Cq, perm = _prep(x, weight, edge_vals, edge_row, edge_col)
    if Cq not in _CACHE:
        _CACHE[Cq] = _build(Cq)
    nc = _CACHE[Cq]

    res = run_bass_kernel_spmd(nc, in_maps, list(range(CORES)))
    LAST_EXEC_TIME_NS = res.exec_time_ns

    out = np.empty((CORES * RPC, D), np.float32)
    for k in range(CORES):
        out[perm[k]] = res.results[k]["outT"].T
    return np.ascontiguousarray(out[:N])
